# revision 31
# baseline (speedup 1.0000x reference)
"""Trainium2 Bass kernel for nn_MultiHeadDynamics.

Computation (per sample row x of state, s of signal):
    heads   = x.reshape(H, DH)                      # H=16, DH=256
    A_h     = U_h @ V_h + diag(d_h)                 # (DH, DH) per head
    lin     = heads @ A_h^T
    c       = heads - mean_dh(heads)
    drift   = lin + cs * c^3 + s
    out     = x + DT*(1+cp)*drift - (DT*cp/H) * sum_h(drift_h)

Folding:  beta = DT*(1+cp);  gp = DT*cp/(H*beta);  q = cbrt(beta*cs)
    D'      = beta*drift
    out     = x + D' - gp * sum_h(D'_h)

Host-side precompute (weight preprocessing, tiny):
    ATm[p, h, k, e] = beta * (A_h[e, d] with d = k*128+p)   as bf16
    Cq[p, k, e]     = q * (I - J/DH)[d, e]                  as bf16
so that on-device, with xT = per-128-chunk PE transpose of x:
    beta*lin (head h) = sum_k  xT_chunk(2h+k).T @ ATm[:, h, k, :]
    ch = q*(x - mean)  (head h) = sum_k  xT_chunk(2h+k).T @ Cq[:, k, :]
    c3 = ch*ch*ch  (fp16, DVE 2x)
    D' in PSUM = beta*lin  (+ c3 + beta*s folded via fp16 identity matmuls)

Sharding: batch B=8192 split across 8 cores (1024 rows each), params
replicated. Per core, rows are processed as 8 tiles of [128, 4096].
"""

import sys

for _p in ("/opt/trn_rl_repo",):
    if _p not in sys.path:
        sys.path.insert(0, _p)

import math
from contextlib import ExitStack

import numpy as np
import ml_dtypes

import concourse.bass as bass
import concourse.tile as tile
from concourse import bacc, mybir
from concourse.bass_utils import run_bass_kernel_spmd
from concourse.masks import make_identity

F32 = mybir.dt.float32
BF16 = mybir.dt.bfloat16
FP16 = mybir.dt.float16
AOP = mybir.AluOpType

# Problem constants (full-input shapes; hardcoded per the task contract).
B = 8192
D = 4096
H = 16
DH = 256
R = 64
DT = 0.05
NCORES = 8
BS = B // NCORES          # rows per core = 1024
P = 128                   # partitions
NT = BS // P              # row tiles per core = 8
NCH = D // P              # 128-wide column chunks per row tile = 32

# Columns of the final fp32 (x + dd) pass handled by DVE; the rest on
# GpSimd.
FINAL_DVE_COLS = 1536


def _emit(tc: tile.TileContext, aps: dict, beta: float, gp: float):
    nc = tc.nc

    state = aps["state"]
    signal = aps["signal"]
    AT_d = aps["ATm"]
    Cq_d = aps["Cq"]
    out_d = aps["out"]

    with ExitStack() as ctx:
        consts = ctx.enter_context(tc.tile_pool(name="consts", bufs=1))

        ident = consts.tile([P, P], F32, tag="ident")
        make_identity(nc, ident)
        identh = consts.tile([P, P], FP16, tag="identh")
        make_identity(nc, identh)

        # --- main loop pools ---
        xp = ctx.enter_context(tc.tile_pool(name="xp", bufs=3))
        sp = ctx.enter_context(tc.tile_pool(name="sp", bufs=3))
        sbp = ctx.enter_context(tc.tile_pool(name="sbp", bufs=2))
        hp = ctx.enter_context(tc.tile_pool(name="hp", bufs=2))
        chp = ctx.enter_context(tc.tile_pool(name="chp", bufs=2))
        c2p = ctx.enter_context(tc.tile_pool(name="c2p", bufs=3))
        c3p = ctx.enter_context(tc.tile_pool(name="c3p", bufs=2))
        ps_tp = ctx.enter_context(tc.tile_pool(name="ps_tp", bufs=2, space="PSUM"))
        ps_ch = ctx.enter_context(tc.tile_pool(name="ps_ch", bufs=2, space="PSUM"))
        ps_lin = ctx.enter_context(tc.tile_pool(name="ps_lin", bufs=2, space="PSUM"))

        AT = consts.tile([P, H * 2 * DH], BF16, tag="AT")
        Cq = consts.tile([P, 2 * DH], BF16, tag="Cq")
        HD = D // 2

        def front(it):
            """Input DMAs, transposes, centering matmuls, cubic, beta*s."""
            r0 = it * P
            st = {}
            # split input streams across the two HWDGE queues (SP / ACT);
            # tile 0's x comes in halves so transposes start sooner.
            # All input triggers ride the idle Sync engine: a DMA trigger
            # executes in its issuing engine's instruction FIFO, so
            # putting it on a busy engine (ACT) delays the transfer start
            # by that engine's backlog. Half-sized transfers interleave
            # x and s on the ring so each consumer starts sooner.
            # Halves ride both HWDGE rings. Input triggers only wait on
            # buffer-frees from 3 tiles ago, so the ACT-issued ones
            # cannot head-block its FIFO the way output triggers would.
            x_t = st["x"] = xp.tile([P, D], F32, tag="x", name="x_t")
            s_t = st["s"] = sp.tile([P, D], F32, tag="s", name="s_t")
            nc.sync.dma_start(out=x_t[:, 0:HD], in_=state[r0:r0 + P, 0:HD])
            nc.scalar.dma_start(out=x_t[:, HD:D], in_=state[r0:r0 + P, HD:D])
            nc.sync.dma_start(out=s_t[:, 0:HD], in_=signal[r0:r0 + P, 0:HD])
            nc.scalar.dma_start(out=s_t[:, HD:D], in_=signal[r0:r0 + P, HD:D])
            sb_t = st["sb"] = sbp.tile([P, D], FP16, tag="sb", name="sb_t")

            if it == 0:
                # Consts ride behind the first input tiles.
                nc.sync.dma_start(out=Cq, in_=Cq_d)
                nc.sync.dma_start(out=AT, in_=AT_d)
                # Warm the PE's HAM clock gate while the first DMA
                # streams so real matmuls run at 2.4 GHz from the start.
                warm = ps_tp.tile([P, 4 * P], F32, tag="tp_ps", name="warm")
                for w in range(16):
                    nc.tensor.matmul(
                        warm[:, (w % 4) * P:(w % 4 + 1) * P], lhsT=ident,
                        rhs=ident, is_transpose=True, skip_group_check=True,
                    )

            # Transpose all 32 f32 chunks of x into hT (d on partitions),
            # casting to bf16 in the PSUM->SBUF copy. Interleave the
            # centering matmuls (ch = q*(x-mean) per head) behind them.
            hT = st["hT"] = hp.tile([P, NCH, P], BF16, tag="hT", name="hT")
            chs = chp.tile([P, D], FP16, tag="chs", name="chs")
            c2_t = st["c2"] = c2p.tile([P, D], FP16, tag="c2", name="c2_t")
            c3_t = st["c3"] = c3p.tile([P, D], FP16, tag="c3", name="c3_t")

            def transp_group(tg):
                tp_ps = ps_tp.tile([P, 4 * P], F32, tag="tp_ps", name="tp_ps")
                for c in range(4):
                    j = tg * 4 + c
                    nc.tensor.transpose(
                        tp_ps[:, c * P:(c + 1) * P],
                        x_t[:, j * P:(j + 1) * P], ident,
                    )
                nc.scalar.copy(
                    out=hT[:, tg * 4:(tg + 1) * 4, :].rearrange(
                        "p a b -> p (a b)"),
                    in_=tp_ps,
                )

            def ch_group(pr):
                # heads 2*pr, 2*pr+1 -> chunks 4*pr .. 4*pr+3
                ch_ps = ps_ch.tile([P, 2 * DH], F32, tag="ch_ps", name="ch_ps")
                # NOTE: start=True clears has_written for the WHOLE bank,
                # so only the first matmul touching a bank may set it.
                for hh in range(2):
                    h = pr * 2 + hh
                    for k in range(2):
                        nc.tensor.matmul(
                            ch_ps[:, hh * DH:(hh + 1) * DH],
                            lhsT=hT[:, 2 * h + k, :],
                            rhs=Cq[:, k * DH:(k + 1) * DH],
                            start=(hh == 0 and k == 0),
                            stop=(hh == 1 and k == 1),
                            skip_group_check=True,
                        )
                nc.scalar.copy(
                    out=chs[:, pr * 2 * DH:(pr + 1) * 2 * DH], in_=ch_ps,
                )

            def cube_half(half):
                # c3 = (q*c)^3 = beta*cs*c^3, fp16 on DVE (2x)
                sl = slice(half * HD, (half + 1) * HD)
                nc.vector.tensor_mul(c2_t[:, sl], chs[:, sl], chs[:, sl])
                nc.vector.tensor_mul(c3_t[:, sl], c2_t[:, sl], chs[:, sl])

            transp_group(0)
            transp_group(1)
            for pr in range(8):
                if pr + 2 < 8:
                    transp_group(pr + 2)
                ch_group(pr)
                if pr in (0, 1):
                    # beta*s in fp16 (fold target for the drift PSUM);
                    # early and per-half so s_t is dead as soon as
                    # possible (its buffer doubles as the out buffer).
                    hs = slice(0, HD) if pr == 0 else slice(HD, D)
                    nc.vector.tensor_scalar(
                        out=sb_t[:, hs], in0=s_t[:, hs], scalar1=beta,
                        scalar2=None, op0=AOP.mult,
                    )
                if pr == 3:
                    cube_half(0)
            cube_half(1)
            st["chs"] = chs
            return st

        def back(it, st):
            """lin matmuls + folds, head-sum coupling, final add, out."""
            r0 = it * P
            x_t, sb_t = st["x"], st["sb"]
            hT, c2_t, c3_t = st["hT"], st["c2"], st["c3"]

            # Per-head-group matmuls into PSUM (beta*lin), then fold
            # c3 and beta*s into the same banks via fp16 identity
            # matmuls -> PSUM holds D' = beta*(lin + cs*c^3 + s).
            # drs reuses the dead chs buffer (cube consumed it in front).
            drs = st["chs"]
            for g in range(4):
                l_ps = ps_lin.tile([P, 4 * DH], F32, tag="l_ps", name="l_ps")
                # one start=True per bank (hh 0 and 2); everything else
                # relies on per-element has_written accumulate-vs-write.
                for hh in range(4):
                    h = g * 4 + hh
                    for k in range(2):
                        nc.tensor.matmul(
                            l_ps[:, hh * DH:(hh + 1) * DH],
                            lhsT=hT[:, 2 * h + k, :],
                            rhs=AT[:, (h * 2 + k) * DH:(h * 2 + k + 1) * DH],
                            start=(hh % 2 == 0 and k == 0), stop=False,
                            skip_group_check=True,
                        )
                for half in range(2):
                    psl = slice(half * 2 * DH, (half + 1) * 2 * DH)
                    csl = slice(g * 4 * DH + half * 2 * DH,
                                g * 4 * DH + (half + 1) * 2 * DH)
                    nc.tensor.matmul(
                        l_ps[:, psl], lhsT=identh, rhs=c3_t[:, csl],
                        start=False, stop=False, skip_group_check=True,
                    )
                    nc.tensor.matmul(
                        l_ps[:, psl], lhsT=identh, rhs=sb_t[:, csl],
                        start=False, stop=True, skip_group_check=True,
                    )
                nc.scalar.copy(out=drs[:, g * 4 * DH:(g + 1) * 4 * DH],
                               in_=l_ps)

            # head-sum tree, flat contiguous halves (order-independent
            # sum). Scratch lives in the dead c3/sb buffers: c3 and sb
            # have been folded into PSUM by now, so reusing them costs
            # no SBUF and the WAR dep is already satisfied.
            t8 = c3_t[:, 0:D // 2]
            nc.vector.tensor_add(t8, drs[:, 0:D // 2], drs[:, D // 2:D])
            t4 = c3_t[:, D // 2:D // 2 + D // 4]
            nc.vector.tensor_add(t4, t8[:, 0:D // 4], t8[:, D // 4:D // 2])
            t2r = c3_t[:, 3 * D // 4:3 * D // 4 + D // 8]
            nc.vector.tensor_add(t2r, t4[:, 0:D // 8], t4[:, D // 8:D // 4])
            # mlt = 4 side-by-side copies of -gp*sum_h(D')
            mlt = sb_t[:, 0:4 * DH]
            nc.vector.tensor_add(mlt[:, 0:DH], t2r[:, 0:DH],
                                 t2r[:, DH:2 * DH])
            nc.vector.tensor_scalar_mul(mlt[:, 0:DH], mlt[:, 0:DH], -gp)
            nc.vector.tensor_copy(mlt[:, DH:2 * DH], mlt[:, 0:DH])
            nc.vector.tensor_copy(mlt[:, 2 * DH:4 * DH], mlt[:, 0:2 * DH])

            # dd = D' + mlt (head-group flat adds; into c2's dead buffer)
            dd_t = c2_t
            for g in range(4):
                gsl = slice(g * 4 * DH, (g + 1) * 4 * DH)
                nc.vector.tensor_add(dd_t[:, gsl], drs[:, gsl], mlt)

            # out = x + dd (fp32+fp16 mixed, split DVE / GpSimd), written
            # into the dead s buffer so x_t is freed by the adds
            # themselves rather than held hostage by the writeback DMA.
            # Last tile goes DVE-heavy: GpSimd's slow fp32 adds would
            # sit on the drain-out critical path.
            o_t = st["s"]
            ncol = 3072 if it == NT - 1 else FINAL_DVE_COLS
            if ncol > 0:
                nc.vector.tensor_add(
                    o_t[:, 0:ncol], x_t[:, 0:ncol], dd_t[:, 0:ncol]
                )
            if ncol < HD:
                # two GpSimd adds so each writeback half can leave as
                # soon as its half of the output is ready
                nc.gpsimd.tensor_add(
                    o_t[:, ncol:HD], x_t[:, ncol:HD], dd_t[:, ncol:HD]
                )
            nc.gpsimd.tensor_add(
                o_t[:, max(ncol, HD):D], x_t[:, max(ncol, HD):D],
                dd_t[:, max(ncol, HD):D]
            )
            # writeback on the GpSimd (SWDGE) queue: a trigger waiting on
            # the final adds would head-block any busier engine's FIFO.
            nc.gpsimd.dma_start(out=out_d[r0:r0 + P, 0:HD],
                                in_=o_t[:, 0:HD])
            nc.gpsimd.dma_start(out=out_d[r0:r0 + P, HD:D],
                                in_=o_t[:, HD:D])

        # Software pipeline: F(0) F(1) B(0) F(2) B(1) ... B(NT-1).
        # Each engine always has a tile's worth of independent work in
        # its queue, so the PE->ACT->DVE->PE dependency ring never
        # head-blocks an engine FIFO.
        pend = []
        for it in range(NT):
            pend.append((it, front(it)))
            if it >= 1:
                bit, bst = pend.pop(0)
                back(bit, bst)
        bit, bst = pend.pop(0)
        back(bit, bst)


_CACHE: dict = {}


def _build(beta: float, gp: float) -> bass.Bass:
    key = (float(beta), float(gp), FINAL_DVE_COLS)
    if key in _CACHE:
        return _CACHE[key]
    nc = bacc.Bacc("TRN2", target_bir_lowering=False, debug=False)
    aps = {
        "state": nc.dram_tensor("state", [BS, D], F32, kind="ExternalInput").ap(),
        "signal": nc.dram_tensor("signal", [BS, D], F32, kind="ExternalInput").ap(),
        "ATm": nc.dram_tensor("ATm", [P, H * 2 * DH], BF16, kind="ExternalInput").ap(),
        "Cq": nc.dram_tensor("Cq", [P, 2 * DH], BF16, kind="ExternalInput").ap(),
        "out": nc.dram_tensor("out", [BS, D], F32, kind="ExternalOutput").ap(),
    }
    with tile.TileContext(nc) as tc:
        _emit(tc, aps, float(beta), float(gp))
    nc.compile()
    _CACHE[key] = nc
    return nc


def _host_params(U, V, diag, cubic_scale, coupling):
    """Fold the tiny per-head params into the matmul operand layouts."""
    beta = DT * (1.0 + coupling)
    gp = DT * coupling / (H * beta)
    q = (beta * cubic_scale) ** (1.0 / 3.0)

    # Reference: A[h, d1, e1] = sum_r U[h,d1,r] V[h,r,e1]; in the lin
    # einsum A is indexed [h, e, d] -> M[h, d, e] := A[h, e, d] (+ diag).
    A = np.einsum("hdr,hre->hde", U, V).astype(np.float32)
    M = np.ascontiguousarray(np.transpose(A, (0, 2, 1)))
    idx = np.arange(DH)
    M[:, idx, idx] += diag
    ATm = (beta * M).reshape(H, 2, P, DH).transpose(2, 0, 1, 3)
    ATm = np.ascontiguousarray(ATm.reshape(P, H * 2 * DH)).astype(
        ml_dtypes.bfloat16
    )

    Cmat = q * (np.eye(DH, dtype=np.float32) - 1.0 / DH)
    Cq = Cmat.reshape(2, P, DH).transpose(1, 0, 2)
    Cq = np.ascontiguousarray(Cq.reshape(P, 2 * DH)).astype(ml_dtypes.bfloat16)
    return beta, gp, ATm, Cq


def run(state, signal, U, V, diag, cubic_scale, coupling, trace=False):
    state = np.ascontiguousarray(np.asarray(state, dtype=np.float32))
    signal = np.ascontiguousarray(np.asarray(signal, dtype=np.float32))
    U = np.asarray(U, dtype=np.float32)
    V = np.asarray(V, dtype=np.float32)
    diag = np.asarray(diag, dtype=np.float32)

    beta, gp, ATm, Cq = _host_params(U, V, diag, float(cubic_scale),
                                     float(coupling))
    nc = _build(beta, gp)
    in_maps = []
    for i in range(NCORES):
        sl = slice(i * BS, (i + 1) * BS)
        in_maps.append({
            "state": state[sl], "signal": signal[sl],
            "ATm": ATm, "Cq": Cq,
        })
    res = run_bass_kernel_spmd(nc, in_maps, list(range(NCORES)), trace=trace)
    out = np.concatenate([res.results[i]["out"] for i in range(NCORES)], axis=0)
    return out, res


def kernel(state, signal, U, V, diag, cubic_scale, coupling) -> np.ndarray:
    out, _ = run(state, signal, U, V, diag, cubic_scale, coupling, trace=False)
    return out


# revision 32
# speedup vs baseline: 1.1258x; 1.1258x over previous
"""Trainium2 Bass kernel for nn_MultiHeadDynamics.

Computation (per sample row x of state, s of signal):
    heads   = x.reshape(H, DH)                      # H=16, DH=256
    A_h     = U_h @ V_h + diag(d_h)                 # (DH, DH) per head
    lin     = heads @ A_h^T
    c       = heads - mean_dh(heads)
    drift   = lin + cs * c^3 + s
    out     = x + DT*(1+cp)*drift - (DT*cp/H) * sum_h(drift_h)

Folding:  beta = DT*(1+cp);  gp = DT*cp/(H*beta);  q = cbrt(beta*cs)
    D'      = beta*drift
    out     = x + D' - gp * sum_h(D'_h)

Host-side precompute (weight preprocessing, tiny):
    ATm[p, h, k, e] = beta * (A_h[e, d] with d = k*128+p)   as bf16
    Cq[p, k, e]     = q * (I - J/DH)[d, e]                  as bf16
so that on-device, with xT = per-128-chunk PE transpose of x:
    beta*lin (head h) = sum_k  xT_chunk(2h+k).T @ ATm[:, h, k, :]
    ch = q*(x - mean)  (head h) = sum_k  xT_chunk(2h+k).T @ Cq[:, k, :]
    c3 = ch*ch*ch  (fp16, DVE 2x)
    D' in PSUM = beta*lin  (+ c3 + beta*s folded via fp16 identity matmuls)

Sharding: batch B=8192 split across 8 cores (1024 rows each), params
replicated. Per core, rows are processed as 8 tiles of [128, 4096].
"""

import sys

for _p in ("/opt/trn_rl_repo",):
    if _p not in sys.path:
        sys.path.insert(0, _p)

import math
from contextlib import ExitStack

import numpy as np
import ml_dtypes

import concourse.bass as bass
import concourse.tile as tile
from concourse import bacc, mybir
from concourse.bass_utils import run_bass_kernel_spmd
from concourse.masks import make_identity

F32 = mybir.dt.float32
BF16 = mybir.dt.bfloat16
FP16 = mybir.dt.float16
AOP = mybir.AluOpType

# Problem constants (full-input shapes; hardcoded per the task contract).
B = 8192
D = 4096
H = 16
DH = 256
R = 64
DT = 0.05
NCORES = 8
BS = B // NCORES          # rows per core = 1024
P = 128                   # partitions
NT = BS // P              # row tiles per core = 8
NCH = D // P              # 128-wide column chunks per row tile = 32

# Columns of the final fp32 (x + dd) pass handled by DVE; the rest on
# GpSimd.
FINAL_DVE_COLS = 1536


def _emit(tc: tile.TileContext, aps: dict, beta: float, gp: float):
    nc = tc.nc

    state = aps["state"]
    signal = aps["signal"]
    AT_d = aps["ATm"]
    Cq_d = aps["Cq"]
    out_d = aps["out"]

    with ExitStack() as ctx:
        consts = ctx.enter_context(tc.tile_pool(name="consts", bufs=1))

        ident = consts.tile([P, P], F32, tag="ident")
        make_identity(nc, ident)
        identh = consts.tile([P, P], FP16, tag="identh")
        make_identity(nc, identh)

        # --- main loop pools ---
        xp = ctx.enter_context(tc.tile_pool(name="xp", bufs=3))
        sp = ctx.enter_context(tc.tile_pool(name="sp", bufs=3))
        sbp = ctx.enter_context(tc.tile_pool(name="sbp", bufs=2))
        hp = ctx.enter_context(tc.tile_pool(name="hp", bufs=2))
        chp = ctx.enter_context(tc.tile_pool(name="chp", bufs=2))
        c2p = ctx.enter_context(tc.tile_pool(name="c2p", bufs=3))
        c3p = ctx.enter_context(tc.tile_pool(name="c3p", bufs=2))
        ps_tp = ctx.enter_context(tc.tile_pool(name="ps_tp", bufs=2, space="PSUM"))
        ps_ch = ctx.enter_context(tc.tile_pool(name="ps_ch", bufs=2, space="PSUM"))
        ps_lin = ctx.enter_context(tc.tile_pool(name="ps_lin", bufs=2, space="PSUM"))

        AT = consts.tile([P, H * 2 * DH], BF16, tag="AT")
        Cq = consts.tile([P, 2 * DH], BF16, tag="Cq")
        HD = D // 2

        def front(it):
            """Input DMAs, transposes, centering matmuls, cubic, beta*s."""
            r0 = it * P
            st = {}
            # split input streams across the two HWDGE queues (SP / ACT);
            # tile 0's x comes in halves so transposes start sooner.
            # All input triggers ride the idle Sync engine: a DMA trigger
            # executes in its issuing engine's instruction FIFO, so
            # putting it on a busy engine (ACT) delays the transfer start
            # by that engine's backlog. Half-sized transfers interleave
            # x and s on the ring so each consumer starts sooner.
            x_t = st["x"] = xp.tile([P, D], F32, tag="x", name="x_t")
            s_t = st["s"] = sp.tile([P, D], F32, tag="s", name="s_t")
            nc.sync.dma_start(out=x_t[:, 0:HD], in_=state[r0:r0 + P, 0:HD])
            nc.sync.dma_start(out=x_t[:, HD:D], in_=state[r0:r0 + P, HD:D])
            nc.sync.dma_start(out=s_t[:, 0:HD], in_=signal[r0:r0 + P, 0:HD])
            nc.sync.dma_start(out=s_t[:, HD:D], in_=signal[r0:r0 + P, HD:D])
            sb_t = st["sb"] = sbp.tile([P, D], FP16, tag="sb", name="sb_t")

            if it == 0:
                # Consts ride behind the first input tiles.
                nc.sync.dma_start(out=Cq, in_=Cq_d)
                nc.sync.dma_start(out=AT, in_=AT_d)
                # Warm the PE's HAM clock gate while the first DMA
                # streams so real matmuls run at 2.4 GHz from the start.
                warm = ps_tp.tile([P, 4 * P], F32, tag="tp_ps", name="warm")
                for w in range(16):
                    nc.tensor.matmul(
                        warm[:, (w % 4) * P:(w % 4 + 1) * P], lhsT=ident,
                        rhs=ident, is_transpose=True, skip_group_check=True,
                    )

            # Transpose all 32 f32 chunks of x into hT (d on partitions),
            # casting to bf16 in the PSUM->SBUF copy. Interleave the
            # centering matmuls (ch = q*(x-mean) per head) behind them.
            hT = st["hT"] = hp.tile([P, NCH, P], BF16, tag="hT", name="hT")
            chs = chp.tile([P, D], FP16, tag="chs", name="chs")
            c2_t = st["c2"] = c2p.tile([P, D], FP16, tag="c2", name="c2_t")
            c3_t = st["c3"] = c3p.tile([P, D], FP16, tag="c3", name="c3_t")

            def transp_group(tg):
                tp_ps = ps_tp.tile([P, 4 * P], F32, tag="tp_ps", name="tp_ps")
                for c in range(4):
                    j = tg * 4 + c
                    nc.tensor.transpose(
                        tp_ps[:, c * P:(c + 1) * P],
                        x_t[:, j * P:(j + 1) * P], ident,
                    )
                nc.scalar.copy(
                    out=hT[:, tg * 4:(tg + 1) * 4, :].rearrange(
                        "p a b -> p (a b)"),
                    in_=tp_ps,
                )

            def ch_group(pr):
                # heads 2*pr, 2*pr+1 -> chunks 4*pr .. 4*pr+3
                ch_ps = ps_ch.tile([P, 2 * DH], F32, tag="ch_ps", name="ch_ps")
                # NOTE: start=True clears has_written for the WHOLE bank,
                # so only the first matmul touching a bank may set it.
                for hh in range(2):
                    h = pr * 2 + hh
                    for k in range(2):
                        nc.tensor.matmul(
                            ch_ps[:, hh * DH:(hh + 1) * DH],
                            lhsT=hT[:, 2 * h + k, :],
                            rhs=Cq[:, k * DH:(k + 1) * DH],
                            start=(hh == 0 and k == 0),
                            stop=(hh == 1 and k == 1),
                            skip_group_check=True,
                        )
                nc.scalar.copy(
                    out=chs[:, pr * 2 * DH:(pr + 1) * 2 * DH], in_=ch_ps,
                )

            def cube_half(half):
                # c3 = (q*c)^3 = beta*cs*c^3, fp16 on DVE (2x)
                sl = slice(half * HD, (half + 1) * HD)
                nc.vector.tensor_mul(c2_t[:, sl], chs[:, sl], chs[:, sl])
                nc.vector.tensor_mul(c3_t[:, sl], c2_t[:, sl], chs[:, sl])

            transp_group(0)
            transp_group(1)
            for pr in range(8):
                if pr + 2 < 8:
                    transp_group(pr + 2)
                ch_group(pr)
                if pr in (0, 1):
                    # beta*s in fp16 (fold target for the drift PSUM);
                    # early and per-half so s_t is dead as soon as
                    # possible (its buffer doubles as the out buffer).
                    hs = slice(0, HD) if pr == 0 else slice(HD, D)
                    nc.vector.tensor_scalar(
                        out=sb_t[:, hs], in0=s_t[:, hs], scalar1=beta,
                        scalar2=None, op0=AOP.mult,
                    )
                if pr == 3:
                    cube_half(0)
            cube_half(1)
            st["chs"] = chs
            return st

        def back(it, st):
            """lin matmuls + folds, head-sum coupling, final add, out."""
            r0 = it * P
            x_t, sb_t = st["x"], st["sb"]
            hT, c2_t, c3_t = st["hT"], st["c2"], st["c3"]

            # Per-head-group matmuls into PSUM (beta*lin), then fold
            # c3 and beta*s into the same banks via fp16 identity
            # matmuls -> PSUM holds D' = beta*(lin + cs*c^3 + s).
            # drs reuses the dead chs buffer (cube consumed it in front).
            drs = st["chs"]
            for g in range(4):
                l_ps = ps_lin.tile([P, 4 * DH], F32, tag="l_ps", name="l_ps")
                # one start=True per bank (hh 0 and 2); everything else
                # relies on per-element has_written accumulate-vs-write.
                for hh in range(4):
                    h = g * 4 + hh
                    for k in range(2):
                        nc.tensor.matmul(
                            l_ps[:, hh * DH:(hh + 1) * DH],
                            lhsT=hT[:, 2 * h + k, :],
                            rhs=AT[:, (h * 2 + k) * DH:(h * 2 + k + 1) * DH],
                            start=(hh % 2 == 0 and k == 0), stop=False,
                            skip_group_check=True,
                        )
                for half in range(2):
                    psl = slice(half * 2 * DH, (half + 1) * 2 * DH)
                    csl = slice(g * 4 * DH + half * 2 * DH,
                                g * 4 * DH + (half + 1) * 2 * DH)
                    nc.tensor.matmul(
                        l_ps[:, psl], lhsT=identh, rhs=c3_t[:, csl],
                        start=False, stop=False, skip_group_check=True,
                    )
                    nc.tensor.matmul(
                        l_ps[:, psl], lhsT=identh, rhs=sb_t[:, csl],
                        start=False, stop=True, skip_group_check=True,
                    )
                nc.scalar.copy(out=drs[:, g * 4 * DH:(g + 1) * 4 * DH],
                               in_=l_ps)

            # head-sum tree, flat contiguous halves (order-independent
            # sum). Scratch lives in the dead c3/sb buffers: c3 and sb
            # have been folded into PSUM by now, so reusing them costs
            # no SBUF and the WAR dep is already satisfied.
            t8 = c3_t[:, 0:D // 2]
            nc.vector.tensor_add(t8, drs[:, 0:D // 2], drs[:, D // 2:D])
            t4 = c3_t[:, D // 2:D // 2 + D // 4]
            nc.vector.tensor_add(t4, t8[:, 0:D // 4], t8[:, D // 4:D // 2])
            t2r = c3_t[:, 3 * D // 4:3 * D // 4 + D // 8]
            nc.vector.tensor_add(t2r, t4[:, 0:D // 8], t4[:, D // 8:D // 4])
            # mlt = 4 side-by-side copies of -gp*sum_h(D')
            mlt = sb_t[:, 0:4 * DH]
            nc.vector.tensor_add(mlt[:, 0:DH], t2r[:, 0:DH],
                                 t2r[:, DH:2 * DH])
            nc.vector.tensor_scalar_mul(mlt[:, 0:DH], mlt[:, 0:DH], -gp)
            nc.vector.tensor_copy(mlt[:, DH:2 * DH], mlt[:, 0:DH])
            nc.vector.tensor_copy(mlt[:, 2 * DH:4 * DH], mlt[:, 0:2 * DH])

            # dd = D' + mlt (head-group flat adds; into c2's dead buffer)
            dd_t = c2_t
            for g in range(4):
                gsl = slice(g * 4 * DH, (g + 1) * 4 * DH)
                nc.vector.tensor_add(dd_t[:, gsl], drs[:, gsl], mlt)

            # out = x + dd (fp32+fp16 mixed, split DVE / GpSimd), written
            # into the dead s buffer so x_t is freed by the adds
            # themselves rather than held hostage by the writeback DMA.
            # Last tile goes DVE-heavy: GpSimd's slow fp32 adds would
            # sit on the drain-out critical path.
            o_t = st["s"]
            ncol = 3072 if it == NT - 1 else FINAL_DVE_COLS
            if ncol > 0:
                nc.vector.tensor_add(
                    o_t[:, 0:ncol], x_t[:, 0:ncol], dd_t[:, 0:ncol]
                )
            if ncol < HD:
                # two GpSimd adds so each writeback half can leave as
                # soon as its half of the output is ready
                nc.gpsimd.tensor_add(
                    o_t[:, ncol:HD], x_t[:, ncol:HD], dd_t[:, ncol:HD]
                )
            nc.gpsimd.tensor_add(
                o_t[:, max(ncol, HD):D], x_t[:, max(ncol, HD):D],
                dd_t[:, max(ncol, HD):D]
            )
            # writeback on the GpSimd (SWDGE) queue: a trigger waiting on
            # the final adds would head-block any busier engine's FIFO.
            nc.gpsimd.dma_start(out=out_d[r0:r0 + P, 0:HD],
                                in_=o_t[:, 0:HD])
            nc.gpsimd.dma_start(out=out_d[r0:r0 + P, HD:D],
                                in_=o_t[:, HD:D])

        # Software pipeline: F(0) F(1) B(0) F(2) B(1) ... B(NT-1).
        # Each engine always has a tile's worth of independent work in
        # its queue, so the PE->ACT->DVE->PE dependency ring never
        # head-blocks an engine FIFO.
        pend = []
        for it in range(NT):
            pend.append((it, front(it)))
            if it >= 1:
                bit, bst = pend.pop(0)
                back(bit, bst)
        bit, bst = pend.pop(0)
        back(bit, bst)


_CACHE: dict = {}


def _build(beta: float, gp: float) -> bass.Bass:
    key = (float(beta), float(gp), FINAL_DVE_COLS)
    if key in _CACHE:
        return _CACHE[key]
    nc = bacc.Bacc("TRN2", target_bir_lowering=False, debug=False)
    aps = {
        "state": nc.dram_tensor("state", [BS, D], F32, kind="ExternalInput").ap(),
        "signal": nc.dram_tensor("signal", [BS, D], F32, kind="ExternalInput").ap(),
        "ATm": nc.dram_tensor("ATm", [P, H * 2 * DH], BF16, kind="ExternalInput").ap(),
        "Cq": nc.dram_tensor("Cq", [P, 2 * DH], BF16, kind="ExternalInput").ap(),
        "out": nc.dram_tensor("out", [BS, D], F32, kind="ExternalOutput").ap(),
    }
    with tile.TileContext(nc) as tc:
        _emit(tc, aps, float(beta), float(gp))
    nc.compile()
    _CACHE[key] = nc
    return nc


def _host_params(U, V, diag, cubic_scale, coupling):
    """Fold the tiny per-head params into the matmul operand layouts."""
    beta = DT * (1.0 + coupling)
    gp = DT * coupling / (H * beta)
    q = (beta * cubic_scale) ** (1.0 / 3.0)

    # Reference: A[h, d1, e1] = sum_r U[h,d1,r] V[h,r,e1]; in the lin
    # einsum A is indexed [h, e, d] -> M[h, d, e] := A[h, e, d] (+ diag).
    A = np.einsum("hdr,hre->hde", U, V).astype(np.float32)
    M = np.ascontiguousarray(np.transpose(A, (0, 2, 1)))
    idx = np.arange(DH)
    M[:, idx, idx] += diag
    ATm = (beta * M).reshape(H, 2, P, DH).transpose(2, 0, 1, 3)
    ATm = np.ascontiguousarray(ATm.reshape(P, H * 2 * DH)).astype(
        ml_dtypes.bfloat16
    )

    Cmat = q * (np.eye(DH, dtype=np.float32) - 1.0 / DH)
    Cq = Cmat.reshape(2, P, DH).transpose(1, 0, 2)
    Cq = np.ascontiguousarray(Cq.reshape(P, 2 * DH)).astype(ml_dtypes.bfloat16)
    return beta, gp, ATm, Cq


def run(state, signal, U, V, diag, cubic_scale, coupling, trace=False):
    state = np.ascontiguousarray(np.asarray(state, dtype=np.float32))
    signal = np.ascontiguousarray(np.asarray(signal, dtype=np.float32))
    U = np.asarray(U, dtype=np.float32)
    V = np.asarray(V, dtype=np.float32)
    diag = np.asarray(diag, dtype=np.float32)

    beta, gp, ATm, Cq = _host_params(U, V, diag, float(cubic_scale),
                                     float(coupling))
    nc = _build(beta, gp)
    in_maps = []
    for i in range(NCORES):
        sl = slice(i * BS, (i + 1) * BS)
        in_maps.append({
            "state": state[sl], "signal": signal[sl],
            "ATm": ATm, "Cq": Cq,
        })
    res = run_bass_kernel_spmd(nc, in_maps, list(range(NCORES)), trace=trace)
    out = np.concatenate([res.results[i]["out"] for i in range(NCORES)], axis=0)
    return out, res


def kernel(state, signal, U, V, diag, cubic_scale, coupling) -> np.ndarray:
    out, _ = run(state, signal, U, V, diag, cubic_scale, coupling, trace=False)
    return out
